# revision 41
# baseline (speedup 1.0000x reference)
"""Trainium2 Bass kernel for a 6-layer post-BatchNorm transformer encoder.

Reference model:
  x = emb[seq] + pes                                  # [B,S,D] = [4,512,1024]
  6x: x = BN(x + attn(x)); x = BN(x + ffn(x))
  BN = per-channel batch stats over (B,S), eps=1e-3.

Sharding: tensor-parallel across 8 NeuronCores (2 heads + 512 FFN hidden per
core). v2 replaces the fp32 AllReduce + redundant-BN design with a chunked
fp16 ReduceScatter -> per-core BN on a 128-channel slice -> chunked fp16
AllGather pipeline: each sublayer's partial output is written in 4 token
chunks (512 tokens = one batch element each); RS chunk c overlaps with
compute of chunk c+1, BN stats accumulate per chunk as RS results land, and
after the affine is finalized the AG chunks stream back while the next
sublayer's matmuls consume them chunk-by-chunk. The residual x/8 is folded
into each partial via a fused DVE scalar_tensor_tensor (no extra PE matmul).

All activations and weights are fp16 (PSUM accumulation fp32; softmax
normalization and BN statistics fp32). Numpy emulation of this exact
quantization scheme gives max rel err ~2.5e-3 vs the fp32 reference
(tolerance 2e-2).

Activation layout: transposed. x^T lives in SBUF as [128 part, 8 dtile,
2048 tok]. Embedding uses dma_gather(transpose=True) which delivers rows
directly in x^T layout (no PE transposes). V is produced token-major by
swapping stationary/moving in the matmul (V_tile = x_tile^T @ Wv), with the
bias added via a ones-row rank-1 matmul, so no V transposes either.
"""

import os

import numpy as np

import concourse.bass as bass
import concourse.mybir as mybir
import concourse.tile as tile
from concourse import bacc
from concourse.bass import ts
from concourse.masks import make_identity

# ---------------------------------------------------------------- dims
V, D, L, H, B, S = 32000, 1024, 6, 16, 4, 512
HD = D // H            # 64
DF = 4 * D             # 4096
EPS = 1e-3
NC = 8                 # cores
T = B * S              # 2048 tokens
P = 128                # partitions
DT = D // P            # 8 d-tiles
TT = T // P            # 16 token tiles
CH = 512               # token chunk (matmul N) == S
NCH = T // CH          # 4 chunks == B
HPC = H // NC          # heads per core = 2
DSH = HPC * HD         # qkv out shard = 128
FSH = DF // NC         # ffn hidden shard = 512
FMT = FSH // P         # ffn1 m-tiles = 4
KL = FSH // P          # ffn2 k-tiles = 4

f32 = mybir.dt.float32
f16 = mybir.dt.float16
f32r = mybir.dt.float32r
i16 = mybir.dt.int16
AF = mybir.ActivationFunctionType
ALU = mybir.AluOpType
AXX = mybir.AxisListType.X

REPLICAS = [list(range(NC))]

N_LAYERS = int(os.environ.get("TRN_KERNEL_LAYERS", str(L)))
GATHER_QUEUES = int(os.environ.get("TRN_GATHER_QUEUES", "1"))


def build_module(n_layers=None):
    if n_layers is None:
        n_layers = N_LAYERS
    nc = bacc.Bacc("TRN2", target_bir_lowering=False, debug=False,
                   num_devices=NC)

    dt_ = nc.dram_tensor
    io = {
        "emb": dt_("emb", [V, D], f16, kind="ExternalInput").ap(),
        "idx": dt_("idx", [16, T // 16], i16, kind="ExternalInput").ap(),
        "pesT": dt_("pesT", [D, S], f16, kind="ExternalInput").ap(),
        "wq": dt_("wq", [L, D, DSH], f16, kind="ExternalInput").ap(),
        "wk": dt_("wk", [L, D, DSH], f16, kind="ExternalInput").ap(),
        "wv": dt_("wv", [L, D, DSH], f16, kind="ExternalInput").ap(),
        "wo": dt_("wo", [L, DSH, D], f16, kind="ExternalInput").ap(),
        "w1": dt_("w1", [L, D, FSH], f16, kind="ExternalInput").ap(),
        "w2": dt_("w2", [L, FSH, D], f16, kind="ExternalInput").ap(),
        "bq": dt_("bq", [L, DSH], f32, kind="ExternalInput").ap(),
        "bk": dt_("bk", [L, DSH], f32, kind="ExternalInput").ap(),
        "bvr": dt_("bvr", [L, 1, DSH], f16, kind="ExternalInput").ap(),
        "b1": dt_("b1", [L, FSH], f32, kind="ExternalInput").ap(),
        "g1s": dt_("g1s", [L, P], f32, kind="ExternalInput").ap(),
        "be1s": dt_("be1s", [L, P], f32, kind="ExternalInput").ap(),
        "g2s": dt_("g2s", [L, P], f32, kind="ExternalInput").ap(),
        "be2s": dt_("be2s", [L, P], f32, kind="ExternalInput").ap(),
        "out": dt_("out", [D, T], f16, kind="ExternalOutput").ap(),
    }

    with tile.TileContext(nc) as tc:
        _build(tc, n_layers, io)
    nc.compile()
    return nc


def _build(tc, n_layers, io):
    from contextlib import ExitStack
    nc = tc.nc
    att_scale = 1.0 / np.sqrt(HD)

    # ------------------------------------------------ pools
    st = ExitStack()
    persist = st.enter_context(tc.tile_pool(name="persist", bufs=1))
    wpool = st.enter_context(tc.tile_pool(name="wpool", bufs=2))   # W1/W2
    wqkv = st.enter_context(tc.tile_pool(name="wqkv", bufs=2))     # Wq/Wk/Wv/Wo
    small = st.enter_context(tc.tile_pool(name="small", bufs=2))   # biases/stats
    ytp = st.enter_context(tc.tile_pool(name="ytp", bufs=2))       # RS result
    e512 = st.enter_context(tc.tile_pool(name="e512", bufs=10))    # [128, CH] f16
    rp = st.enter_context(tc.tile_pool(name="rp", bufs=2))         # f32 recips
    htp = st.enter_context(tc.tile_pool(name="htp", bufs=2))       # ffn hidden

    ps = st.enter_context(tc.tile_pool(name="ps", bufs=5, space="PSUM"))
    pst = st.enter_context(tc.tile_pool(name="pst", bufs=2, space="PSUM"))
    crin = st.enter_context(tc.tile_pool(name="crin", bufs=8, space="DRAM"))
    crso = st.enter_context(tc.tile_pool(name="crso", bufs=8, space="DRAM"))
    cagi = st.enter_context(tc.tile_pool(name="cagi", bufs=8, space="DRAM"))
    cago = st.enter_context(tc.tile_pool(name="cago", bufs=8, space="DRAM"))

    # ------------------------------------------------ persistent tiles
    xA = persist.tile([P, DT, T], f16, name="xA")          # x
    xB = persist.tile([P, DT, T], f16, name="xB")          # x2
    qT = persist.tile([P, T], f16, name="qT")              # Q^T shard
    kT = persist.tile([P, T], f16, name="kT")              # K^T shard
    vsb = persist.tile([P, TT, 2 * (HD + 1)], f16, name="vsb")  # [V|1|V|1]
    onesH = persist.tile([P, P], f16, name="onesH")
    attnTA = persist.tile([HD, T], f16, name="attnTA")     # head-0 attn^T
    attnTB = persist.tile([HD, T], f16, name="attnTB")     # head-1 attn^T
    pes_sb = persist.tile([P, DT, S], f16, name="pes_sb")  # pes^T
    idxs = persist.tile([P, T // 16], i16, name="idxs")

    nc.vector.memset(onesH[:], 1.0)
    # ones columns in vsb (positions HD and 2*HD+1 of each token tile)
    nc.vector.memset(vsb[:, :, HD:HD + 1], 1.0)
    nc.vector.memset(vsb[:, :, 2 * HD + 1:], 1.0)
    # indices wrapped in 16 partitions, replicated into all 8 Q7-core stripes
    for r_ in range(P // 16):
        nc.sync.dma_start(idxs[16 * r_:16 * (r_ + 1), :], io["idx"])
    nc.sync.dma_start(pes_sb[:], io["pesT"].rearrange("(k p) s -> p k s", p=P))

    # ---------------- embedding: x^T via transposing gather, + pes^T
    for c in range(NCH):
        gt = ytp.tile([P, DT, CH], f16, tag="gt", name=f"gt{c}")
        nc.gpsimd.dma_gather(
            out_ap=gt[:],
            in_ap=io["emb"],
            idxs_ap=idxs[:, c * (CH // 16):(c + 1) * (CH // 16)],
            num_idxs=CH,
            num_idxs_reg=CH,
            elem_size=D,
            transpose=True,
            queue_num=c % GATHER_QUEUES,
        )
        nc.vector.tensor_tensor(out=xA[:, :, ts(c, CH)],
                                in0=gt[:],
                                in1=pes_sb[:], op=ALU.add)

    # ---------------- chunked RS -> per-core-slice BN -> chunked AG
    # Asymmetric splits: big RS first so the last (exposed) RS is small;
    # small AG first so the consumer's first chunk arrives early.
    RS_SPLIT = [CH, CH, CH, CH // 2, CH // 2]   # last chunk split in half
    AG_SPLIT = [CH, 3 * CH]         # token spans per AllGather
    NRS = len(RS_SPLIT)
    rs_chunk_map = {}               # token chunk -> (rs index, col offset)
    _c = 0
    for _ri, _span in enumerate(RS_SPLIT):
        for _j in range(_span // CH):
            rs_chunk_map[_c] = (_ri, _j * CH)
            _c += 1

    def bn_sublayer(lbl, arins, g_sb, be_sb, xout):
        """arins: list of written [D, span] f16 DRAM tiles (partial + x/8).
        Reduces across cores, BN-normalizes this core's 128-channel slice,
        gathers normalized chunks into xout ([P, DT, T] f16)."""
        ysb = ytp.tile([P, T], f16, tag="ysb", name=f"ysb{lbl}")
        ysum = small.tile([P, NRS], f32, tag="ysum", name=f"ysum{lbl}")
        sqp = small.tile([P, NRS], f32, tag="sqp", name=f"sqp{lbl}")
        rof = 0
        for c, span in enumerate(RS_SPLIT):
            rso = crso.tile([P, span], f16, tag=f"rso{c}",
                            name=f"rso{lbl}_{c}")
            nc.gpsimd.collective_compute(
                "ReduceScatter", ALU.add, replica_groups=REPLICAS,
                ins=[arins[c].opt()], outs=[rso.opt()])
            nc.sync.dma_start(ysb[:, rof:rof + span], rso[:])
            nc.vector.reduce_sum(out=ysum[:, c:c + 1],
                                 in_=ysb[:, rof:rof + span], axis=AXX)
            scr = ytp.tile([P, 3 * CH], f16, tag="scr", name=f"sq{lbl}_{c}")
            nc.scalar.activation(scr[:, 0:span], ysb[:, rof:rof + span],
                                 AF.Square, accum_out=sqp[:, c:c + 1])
            rof += span
        stot = small.tile([P, 1], f32, tag="stot", name=f"stot{lbl}")
        sqt = small.tile([P, 1], f32, tag="sqt", name=f"sqt{lbl}")
        nc.vector.reduce_sum(out=stot[:], in_=ysum[:], axis=AXX)
        nc.vector.reduce_sum(out=sqt[:], in_=sqp[:], axis=AXX)
        mean = small.tile([P, 1], f32, tag="mean", name=f"mean{lbl}")
        nc.vector.tensor_scalar_mul(mean[:], stot[:], 1.0 / T)
        msq = small.tile([P, 1], f32, tag="msq", name=f"msq{lbl}")
        nc.vector.tensor_tensor(out=msq[:], in0=mean[:], in1=mean[:],
                                op=ALU.mult)
        veps = small.tile([P, 1], f32, tag="veps", name=f"veps{lbl}")
        nc.vector.scalar_tensor_tensor(out=veps[:], in0=sqt[:], scalar=1.0 / T,
                                       in1=msq[:], op0=ALU.mult,
                                       op1=ALU.subtract)
        nc.vector.tensor_scalar_add(veps[:], veps[:], EPS)
        rec = small.tile([P, 1], f32, tag="rec", name=f"rec{lbl}")
        nc.vector.reciprocal(rec[:], veps[:])
        rstd = small.tile([P, 1], f32, tag="rstd", name=f"rstd{lbl}")
        nc.scalar.sqrt(rstd[:], rec[:])
        sca = small.tile([P, 1], f32, tag="sca", name=f"sca{lbl}")
        nc.vector.tensor_tensor(out=sca[:], in0=g_sb[:], in1=rstd[:],
                                op=ALU.mult)
        sh = small.tile([P, 1], f32, tag="sh", name=f"sh{lbl}")
        nc.vector.tensor_tensor(out=sh[:], in0=mean[:], in1=sca[:],
                                op=ALU.mult)
        nc.vector.tensor_tensor(out=sh[:], in0=be_sb[:], in1=sh[:],
                                op=ALU.subtract)
        aof = 0
        for a, span in enumerate(AG_SPLIT):
            asb = ytp.tile([P, 3 * CH], f16, tag="scr", name=f"ap{lbl}_{a}")
            nc.scalar.activation(asb[:, 0:span], ysb[:, aof:aof + span],
                                 AF.Identity,
                                 bias=sh[:, 0:1], scale=sca[:, 0:1])
            agi = cagi.tile([P, span], f16, tag=f"agi{a}",
                            name=f"agi{lbl}_{a}")
            nc.sync.dma_start(agi[:], asb[:, 0:span])
            ago = cago.tile([D, span], f16, tag=f"ago{a}",
                            addr_space="Shared", name=f"ago{lbl}_{a}")
            nc.gpsimd.collective_compute(
                "AllGather", ALU.bypass, replica_groups=REPLICAS,
                ins=[agi.opt()], outs=[ago.opt()])
            # chunked writeback so the consumer can start on the first chunk
            for cc_ in range(span // CH):
                c_ = aof // CH + cc_
                nc.sync.dma_start(
                    xout[:, :, ts(c_, CH)],
                    ago[:, ts(cc_, CH)].rearrange("(k p) t -> p k t", p=P))
            aof += span

    # ---------------------------------------- layers
    for l in range(n_layers):
        # ---- layer weights/params to SBUF
        wq_sb = wqkv.tile([P, DT, DSH], f16, tag="wq", name=f"wq{l}")
        wk_sb = wqkv.tile([P, DT, DSH], f16, tag="wk", name=f"wk{l}")
        wv_sb = wqkv.tile([P, DT, DSH], f16, tag="wv", name=f"wv{l}")
        wo_sbA = wqkv.tile([HD, D], f16, tag="woA", name=f"woA{l}")
        wo_sbB = wqkv.tile([HD, D], f16, tag="woB", name=f"woB{l}")
        w1_sb = wpool.tile([P, DT, FSH], f16, tag="w1", name=f"w1{l}")
        w2_sb = wpool.tile([P, KL, D], f16, tag="w2", name=f"w2{l}")
        nc.sync.dma_start(wq_sb[:], io["wq"][l].rearrange("(k p) m -> p k m", p=P))
        nc.sync.dma_start(wk_sb[:], io["wk"][l].rearrange("(k p) m -> p k m", p=P))
        nc.sync.dma_start(wv_sb[:], io["wv"][l].rearrange("(k p) m -> p k m", p=P))
        nc.sync.dma_start(wo_sbA[:], io["wo"][l][0:HD, :])
        nc.sync.dma_start(wo_sbB[:], io["wo"][l][HD:2 * HD, :])
        nc.sync.dma_start(w1_sb[:], io["w1"][l].rearrange("(k p) m -> p k m", p=P))
        nc.sync.dma_start(w2_sb[:], io["w2"][l].rearrange("(k p) m -> p k m", p=P))

        bq_sb = small.tile([P, 1], f32, tag="bq", name=f"bq{l}")
        bk_sb = small.tile([P, 1], f32, tag="bk", name=f"bk{l}")
        bvr_sb = small.tile([1, DSH], f16, tag="bvr", name=f"bvr{l}")
        b1_sb = small.tile([P, FMT], f32, tag="b1", name=f"b1{l}")
        nc.sync.dma_start(bq_sb[:], io["bq"][l].rearrange("(p o) -> p o", o=1))
        nc.sync.dma_start(bk_sb[:], io["bk"][l].rearrange("(p o) -> p o", o=1))
        nc.sync.dma_start(bvr_sb[:], io["bvr"][l])
        nc.sync.dma_start(b1_sb[:], io["b1"][l].rearrange("(m p) -> p m", p=P))

        g1_sb = small.tile([P, 1], f32, tag="g1", name=f"g1{l}")
        be1_sb = small.tile([P, 1], f32, tag="be1", name=f"be1{l}")
        g2_sb = small.tile([P, 1], f32, tag="g2", name=f"g2{l}")
        be2_sb = small.tile([P, 1], f32, tag="be2", name=f"be2{l}")
        nc.sync.dma_start(g1_sb[:], io["g1s"][l].rearrange("(p o) -> p o", o=1))
        nc.sync.dma_start(be1_sb[:], io["be1s"][l].rearrange("(p o) -> p o", o=1))
        nc.sync.dma_start(g2_sb[:], io["g2s"][l].rearrange("(p o) -> p o", o=1))
        nc.sync.dma_start(be2_sb[:], io["be2s"][l].rearrange("(p o) -> p o", o=1))

        # ---- attention sublayer: QKV + attn + Wo partials, per token chunk
        arins1 = []
        for c in range(NCH):
            # Q^T/K^T shard for chunk c
            psq = ps.tile([P, CH], f32, tag="mm", name=f"psq{l}_{c}")
            psk = ps.tile([P, CH], f32, tag="mm", name=f"psk{l}_{c}")
            for k in range(DT):
                fl, ll = (k == 0), (k == DT - 1)
                rhs = xA[:, k, ts(c, CH)]
                nc.tensor.matmul(psq[:], wq_sb[:, k, :], rhs, start=fl, stop=ll)
                nc.tensor.matmul(psk[:], wk_sb[:, k, :], rhs, start=fl, stop=ll)
            nc.scalar.activation(qT[:, ts(c, CH)], psq[:], AF.Identity,
                                 bias=bq_sb[:])
            nc.scalar.activation(kT[:, ts(c, CH)], psk[:], AF.Identity,
                                 bias=bk_sb[:])
            # V token-major: V_tile = x_tile^T @ Wv + ones^T bvr
            for tt in range(CH // P):
                t = c * (CH // P) + tt
                psv = pst.tile([P, DSH], f32, tag="tv", name=f"psv{l}_{t}")
                for k in range(DT):
                    nc.tensor.matmul(psv[:], xA[:, k, ts(t, P)], wv_sb[:, k, :],
                                     start=(k == 0), stop=False)
                nc.tensor.matmul(psv[:], onesH[0:1, 0:P], bvr_sb[0:1, :],
                                 start=False, stop=True)
                nc.vector.tensor_copy(
                    vsb[:, t, :].rearrange("p (h x) -> p h x", h=2)[:, :, 0:HD],
                    psv[:].rearrange("p (h x) -> p h x", h=2))

            # attention for batch b == c (keys/values = this chunk)
            b = c
            for h, attnT_h in enumerate([attnTA, attnTB]):
                hp = h * HD
                vof = h * (HD + 1)
                ets = []
                for sk in range(B):
                    pss = ps.tile([P, CH], f32, tag="mm",
                                  name=f"pss{l}_{b}_{h}_{sk}")
                    nc.tensor.matmul(
                        pss[:],
                        kT[hp:hp + HD, b * CH + sk * P:b * CH + (sk + 1) * P],
                        qT[hp:hp + HD, ts(b, CH)],
                        start=True, stop=True)
                    et = e512.tile([P, CH], f16, tag="e",
                                   name=f"et{l}_{b}_{h}_{sk}")
                    nc.scalar.activation(et[:], pss[:], AF.Exp,
                                         scale=att_scale)
                    ets.append(et)
                psu = ps.tile([P, CH], f32, tag="mm", name=f"psu{l}_{b}_{h}")
                for sk in range(B):
                    nc.tensor.matmul(psu[0:HD + 1, :],
                                     vsb[:, b * 4 + sk, vof:vof + HD + 1],
                                     ets[sk][:],
                                     start=(sk == 0), stop=(sk == B - 1))
                rss = rp.tile([1, CH], f32, tag="rss", name=f"rss{l}_{b}_{h}")
                nc.scalar.copy(rss[:], psu[HD:HD + 1, :])
                rsb = rp.tile([1, CH], f32, tag="r32", name=f"rsb{l}_{b}_{h}")
                nc.vector.reciprocal_approx_fast(rsb[:], rss[:])
                rsb16 = rp.tile([1, CH], f16, tag="r16", name=f"rsc{l}_{b}_{h}")
                nc.scalar.copy(rsb16[:], rsb[:])
                psr = ps.tile([P, CH], f32, tag="mm", name=f"psr{l}_{b}_{h}")
                nc.tensor.matmul(psr[0:HD, :], onesH[0:1, 0:HD], rsb16[:],
                                 start=True, stop=True)
                usb = e512.tile([P, CH], f16, tag="e", name=f"usb{l}_{b}_{h}")
                nc.scalar.copy(usb[0:HD, :], psu[0:HD, :])
                nc.vector.tensor_tensor(out=attnT_h[:, ts(b, CH)],
                                        in0=usb[0:HD, :],
                                        in1=psr[0:HD, :], op=ALU.mult)

            # Wo partial + residual/8 for chunk c; last chunk in halves so
            # the final (exposed) ReduceScatter is small and starts early
            subs = ([(c, 0, CH)] if c < NCH - 1 else
                    [(c, 0, CH // 2), (c + 1, CH // 2, CH // 2)])
            for ri, hof, hw in subs:
                arin = crin.tile([D, hw], f16, tag=f"ari{ri}",
                                 name=f"ari1_{l}_{ri}")
                arins1.append(arin)
                for m in range(DT):
                    ps2 = ps.tile([P, CH], f32, tag="mm",
                                  name=f"pso{l}_{m}_{ri}")
                    tk = slice(c * CH + hof, c * CH + hof + hw)
                    nc.tensor.matmul(ps2[:, 0:hw], wo_sbA[:, ts(m, P)],
                                     attnTA[:, tk], start=True, stop=False)
                    nc.tensor.matmul(ps2[:, 0:hw], wo_sbB[:, ts(m, P)],
                                     attnTB[:, tk], start=False, stop=True)
                    osb = e512.tile([P, CH], f16, tag="e",
                                    name=f"osb{l}_{m}_{ri}")
                    nc.vector.scalar_tensor_tensor(
                        out=osb[:, 0:hw], in0=xA[:, m, tk], scalar=1.0 / NC,
                        in1=ps2[:, 0:hw], op0=ALU.mult, op1=ALU.add)
                    nc.sync.dma_start(arin[ts(m, P), :], osb[:, 0:hw])

        # ---- BN1 -> x2 (xB)
        bn_sublayer(f"a{l}", arins1, g1_sb, be1_sb, xB)

        # ---- FFN sublayer, per token chunk
        arins2 = []
        for c in range(NCH):
            ht = htp.tile([P, FMT, CH], f16, tag="ht", name=f"ht{l}_{c}")
            for m in range(FMT):
                ps1 = ps.tile([P, CH], f32, tag="mm", name=f"ps1{l}_{c}_{m}")
                for k in range(DT):
                    nc.tensor.matmul(ps1[:], w1_sb[:, k, ts(m, P)],
                                     xB[:, k, ts(c, CH)],
                                     start=(k == 0), stop=(k == DT - 1))
                nc.scalar.activation(ht[:, m, :], ps1[:], AF.Relu,
                                     bias=b1_sb[:, m:m + 1])
            subs = ([(c, 0, CH)] if c < NCH - 1 else
                    [(c, 0, CH // 2), (c + 1, CH // 2, CH // 2)])
            for ri, hof, hw in subs:
                arin = crin.tile([D, hw], f16, tag=f"ari{ri}",
                                 name=f"ari2_{l}_{ri}")
                arins2.append(arin)
                for m in range(DT):
                    ps2 = ps.tile([P, CH], f32, tag="mm",
                                  name=f"ps2{l}_{ri}_{m}")
                    for k in range(KL):
                        nc.tensor.matmul(ps2[:, 0:hw], w2_sb[:, k, ts(m, P)],
                                         ht[:, k, hof:hof + hw],
                                         start=(k == 0), stop=(k == KL - 1))
                    osb = e512.tile([P, CH], f16, tag="e",
                                    name=f"fsb{l}_{ri}_{m}")
                    tk = slice(c * CH + hof, c * CH + hof + hw)
                    nc.vector.scalar_tensor_tensor(
                        out=osb[:, 0:hw], in0=xB[:, m, tk], scalar=1.0 / NC,
                        in1=ps2[:, 0:hw], op0=ALU.mult, op1=ALU.add)
                    nc.sync.dma_start(arin[ts(m, P), :], osb[:, 0:hw])

        # ---- BN2 -> x(l+1) (xA)
        bn_sublayer(f"f{l}", arins2, g2_sb, be2_sb, xA)

    # ---------------------------------------- output x^T -> [D, T] f16
    nc.sync.dma_start(io["out"].rearrange("(k p) t -> p k t", p=P), xA[:])
    st.close()


# ================================================================ host side

def make_in_maps(inputs):
    f = lambda a: np.ascontiguousarray(np.asarray(a), dtype=np.float32)
    h = lambda a: np.ascontiguousarray(np.asarray(a), dtype=np.float16)
    seq = np.asarray(inputs["sequence"]).reshape(-1).astype(np.int16)
    idx = np.ascontiguousarray(seq.reshape(T // 16, 16).T)     # [16, T//16]
    emb = h(inputs["emb"])
    pesT = np.ascontiguousarray(h(inputs["pes"]).T)            # [D, S]
    Wq, Wk, Wv = h(inputs["Wq"]), h(inputs["Wk"]), h(inputs["Wv"])
    Wo, W1, W2 = h(inputs["Wo"]), h(inputs["W1"]), h(inputs["W2"])
    bq, bk, bv = f(inputs["bq"]), f(inputs["bk"]), f(inputs["bv"])
    b1 = f(inputs["b1"])
    g1, be1 = f(inputs["g1"]), f(inputs["be1"])
    g2, be2 = f(inputs["g2"]), f(inputs["be2"])

    in_maps = []
    for c in range(NC):
        ds_ = slice(c * DSH, (c + 1) * DSH)
        fs_ = slice(c * FSH, (c + 1) * FSH)
        cs_ = slice(c * P, (c + 1) * P)
        in_maps.append({
            "emb": emb,
            "idx": idx,
            "pesT": pesT,
            "wq": np.ascontiguousarray(Wq[:, :, ds_]),
            "wk": np.ascontiguousarray(Wk[:, :, ds_]),
            "wv": np.ascontiguousarray(Wv[:, :, ds_]),
            "wo": np.ascontiguousarray(Wo[:, ds_, :]),
            "w1": np.ascontiguousarray(W1[:, :, fs_]),
            "w2": np.ascontiguousarray(W2[:, fs_, :]),
            "bq": np.ascontiguousarray(bq[:, ds_]),
            "bk": np.ascontiguousarray(bk[:, ds_]),
            "bvr": np.ascontiguousarray(bv[:, ds_]).astype(np.float16)
                     .reshape(L, 1, DSH),
            "b1": np.ascontiguousarray(b1[:, fs_]),
            "g1s": np.ascontiguousarray(g1[:, cs_]),
            "be1s": np.ascontiguousarray(be1[:, cs_]),
            "g2s": np.ascontiguousarray(g2[:, cs_]),
            "be2s": np.ascontiguousarray(be2[:, cs_]),
        })
    return in_maps


_CACHE = {}


def _get_module():
    if "nc" not in _CACHE:
        _CACHE["nc"] = build_module()
    return _CACHE["nc"]


def kernel(**inputs):
    from concourse import bass_utils
    nc = _get_module()
    in_maps = make_in_maps(inputs)
    res = bass_utils.run_bass_kernel_spmd(nc, in_maps, list(range(NC)))
    o = np.asarray(res.results[0]["out"])                  # [D, T]
    return np.ascontiguousarray(o.T).reshape(B, S, D).astype(np.float32)


# revision 42
# speedup vs baseline: 1.1168x; 1.1168x over previous
"""Trainium2 Bass kernel for a 6-layer post-BatchNorm transformer encoder.

Reference model:
  x = emb[seq] + pes                                  # [B,S,D] = [4,512,1024]
  6x: x = BN(x + attn(x)); x = BN(x + ffn(x))
  BN = per-channel batch stats over (B,S), eps=1e-3.

Sharding: tensor-parallel across 8 NeuronCores (2 heads + 512 FFN hidden per
core). v2 replaces the fp32 AllReduce + redundant-BN design with a chunked
fp16 ReduceScatter -> per-core BN on a 128-channel slice -> chunked fp16
AllGather pipeline: each sublayer's partial output is written in 4 token
chunks (512 tokens = one batch element each); RS chunk c overlaps with
compute of chunk c+1, BN stats accumulate per chunk as RS results land, and
after the affine is finalized the AG chunks stream back while the next
sublayer's matmuls consume them chunk-by-chunk. The residual x/8 is folded
into each partial via a fused DVE scalar_tensor_tensor (no extra PE matmul).

All activations and weights are fp16 (PSUM accumulation fp32; softmax
normalization and BN statistics fp32). Numpy emulation of this exact
quantization scheme gives max rel err ~2.5e-3 vs the fp32 reference
(tolerance 2e-2).

Activation layout: transposed. x^T lives in SBUF as [128 part, 8 dtile,
2048 tok]. Embedding uses dma_gather(transpose=True) which delivers rows
directly in x^T layout (no PE transposes). V is produced token-major by
swapping stationary/moving in the matmul (V_tile = x_tile^T @ Wv), with the
bias added via a ones-row rank-1 matmul, so no V transposes either.
"""

import os

import numpy as np

import concourse.bass as bass
import concourse.mybir as mybir
import concourse.tile as tile
from concourse import bacc
from concourse.bass import ts
from concourse.masks import make_identity

# ---------------------------------------------------------------- dims
V, D, L, H, B, S = 32000, 1024, 6, 16, 4, 512
HD = D // H            # 64
DF = 4 * D             # 4096
EPS = 1e-3
NC = 8                 # cores
T = B * S              # 2048 tokens
P = 128                # partitions
DT = D // P            # 8 d-tiles
TT = T // P            # 16 token tiles
CH = 512               # token chunk (matmul N) == S
NCH = T // CH          # 4 chunks == B
HPC = H // NC          # heads per core = 2
DSH = HPC * HD         # qkv out shard = 128
FSH = DF // NC         # ffn hidden shard = 512
FMT = FSH // P         # ffn1 m-tiles = 4
KL = FSH // P          # ffn2 k-tiles = 4

f32 = mybir.dt.float32
f16 = mybir.dt.float16
f32r = mybir.dt.float32r
i16 = mybir.dt.int16
AF = mybir.ActivationFunctionType
ALU = mybir.AluOpType
AXX = mybir.AxisListType.X

REPLICAS = [list(range(NC))]

N_LAYERS = int(os.environ.get("TRN_KERNEL_LAYERS", str(L)))
GATHER_QUEUES = int(os.environ.get("TRN_GATHER_QUEUES", "1"))


def build_module(n_layers=None):
    if n_layers is None:
        n_layers = N_LAYERS
    nc = bacc.Bacc("TRN2", target_bir_lowering=False, debug=False,
                   num_devices=NC)

    dt_ = nc.dram_tensor
    io = {
        "emb": dt_("emb", [V, D], f16, kind="ExternalInput").ap(),
        "idx": dt_("idx", [16, T // 16], i16, kind="ExternalInput").ap(),
        "pesT": dt_("pesT", [D, S], f16, kind="ExternalInput").ap(),
        "wq": dt_("wq", [L, D, DSH], f16, kind="ExternalInput").ap(),
        "wk": dt_("wk", [L, D, DSH], f16, kind="ExternalInput").ap(),
        "wv": dt_("wv", [L, D, DSH], f16, kind="ExternalInput").ap(),
        "wo": dt_("wo", [L, DSH, D], f16, kind="ExternalInput").ap(),
        "w1": dt_("w1", [L, D, FSH], f16, kind="ExternalInput").ap(),
        "w2": dt_("w2", [L, FSH, D], f16, kind="ExternalInput").ap(),
        "bq": dt_("bq", [L, DSH], f32, kind="ExternalInput").ap(),
        "bk": dt_("bk", [L, DSH], f32, kind="ExternalInput").ap(),
        "bvr": dt_("bvr", [L, 1, DSH], f16, kind="ExternalInput").ap(),
        "b1": dt_("b1", [L, FSH], f32, kind="ExternalInput").ap(),
        "g1s": dt_("g1s", [L, P], f32, kind="ExternalInput").ap(),
        "be1s": dt_("be1s", [L, P], f32, kind="ExternalInput").ap(),
        "g2s": dt_("g2s", [L, P], f32, kind="ExternalInput").ap(),
        "be2s": dt_("be2s", [L, P], f32, kind="ExternalInput").ap(),
        "out": dt_("out", [D, T], f16, kind="ExternalOutput").ap(),
    }

    with tile.TileContext(nc) as tc:
        _build(tc, n_layers, io)
    nc.compile()
    return nc


def _build(tc, n_layers, io):
    from contextlib import ExitStack
    nc = tc.nc
    att_scale = 1.0 / np.sqrt(HD)

    # ------------------------------------------------ pools
    st = ExitStack()
    persist = st.enter_context(tc.tile_pool(name="persist", bufs=1))
    wpool = st.enter_context(tc.tile_pool(name="wpool", bufs=2))   # W1/W2
    wqkv = st.enter_context(tc.tile_pool(name="wqkv", bufs=2))     # Wq/Wk/Wv/Wo
    small = st.enter_context(tc.tile_pool(name="small", bufs=2))   # biases/stats
    ytp = st.enter_context(tc.tile_pool(name="ytp", bufs=2))       # RS result
    e512 = st.enter_context(tc.tile_pool(name="e512", bufs=10))    # [128, CH] f16
    rp = st.enter_context(tc.tile_pool(name="rp", bufs=2))         # f32 recips
    htp = st.enter_context(tc.tile_pool(name="htp", bufs=2))       # ffn hidden

    ps = st.enter_context(tc.tile_pool(name="ps", bufs=5, space="PSUM"))
    pst = st.enter_context(tc.tile_pool(name="pst", bufs=2, space="PSUM"))
    crin = st.enter_context(tc.tile_pool(name="crin", bufs=8, space="DRAM"))
    crso = st.enter_context(tc.tile_pool(name="crso", bufs=8, space="DRAM"))
    cagi = st.enter_context(tc.tile_pool(name="cagi", bufs=8, space="DRAM"))
    cago = st.enter_context(tc.tile_pool(name="cago", bufs=8, space="DRAM"))

    # ------------------------------------------------ persistent tiles
    xA = persist.tile([P, DT, T], f16, name="xA")          # x
    xB = persist.tile([P, DT, T], f16, name="xB")          # x2
    qT = persist.tile([P, T], f16, name="qT")              # Q^T shard
    kT = persist.tile([P, T], f16, name="kT")              # K^T shard
    vsb = persist.tile([P, TT, 2 * (HD + 1)], f16, name="vsb")  # [V|1|V|1]
    onesH = persist.tile([P, P], f16, name="onesH")
    attnTA = persist.tile([HD, T], f16, name="attnTA")     # head-0 attn^T
    attnTB = persist.tile([HD, T], f16, name="attnTB")     # head-1 attn^T
    pes_sb = persist.tile([P, DT, S], f16, name="pes_sb")  # pes^T
    idxs = persist.tile([P, T // 16], i16, name="idxs")

    nc.vector.memset(onesH[:], 1.0)
    # ones columns in vsb (positions HD and 2*HD+1 of each token tile)
    nc.vector.memset(vsb[:, :, HD:HD + 1], 1.0)
    nc.vector.memset(vsb[:, :, 2 * HD + 1:], 1.0)
    # indices wrapped in 16 partitions, replicated into all 8 Q7-core stripes
    for r_ in range(P // 16):
        nc.sync.dma_start(idxs[16 * r_:16 * (r_ + 1), :], io["idx"])
    nc.sync.dma_start(pes_sb[:], io["pesT"].rearrange("(k p) s -> p k s", p=P))

    # ---------------- embedding: x^T via transposing gather, + pes^T
    for c in range(NCH):
        gt = ytp.tile([P, DT, CH], f16, tag="gt", name=f"gt{c}")
        nc.gpsimd.dma_gather(
            out_ap=gt[:],
            in_ap=io["emb"],
            idxs_ap=idxs[:, c * (CH // 16):(c + 1) * (CH // 16)],
            num_idxs=CH,
            num_idxs_reg=CH,
            elem_size=D,
            transpose=True,
            queue_num=c % GATHER_QUEUES,
        )
        nc.vector.tensor_tensor(out=xA[:, :, ts(c, CH)],
                                in0=gt[:],
                                in1=pes_sb[:], op=ALU.add)

    # ---------------- chunked RS -> per-core-slice BN -> chunked AG
    # Asymmetric splits: big RS first so the last (exposed) RS is small;
    # small AG first so the consumer's first chunk arrives early.
    RS_SPLIT = [CH, CH, CH, CH]     # token spans per ReduceScatter
    AG_SPLIT = [CH, 3 * CH]         # token spans per AllGather
    NRS = len(RS_SPLIT)
    rs_chunk_map = {}               # token chunk -> (rs index, col offset)
    _c = 0
    for _ri, _span in enumerate(RS_SPLIT):
        for _j in range(_span // CH):
            rs_chunk_map[_c] = (_ri, _j * CH)
            _c += 1

    def bn_sublayer(lbl, arins, g_sb, be_sb, xout):
        """arins: list of written [D, span] f16 DRAM tiles (partial + x/8).
        Reduces across cores, BN-normalizes this core's 128-channel slice,
        gathers normalized chunks into xout ([P, DT, T] f16)."""
        ysb = ytp.tile([P, T], f16, tag="ysb", name=f"ysb{lbl}")
        ysum = small.tile([P, NRS], f32, tag="ysum", name=f"ysum{lbl}")
        sqp = small.tile([P, NRS], f32, tag="sqp", name=f"sqp{lbl}")
        rof = 0
        for c, span in enumerate(RS_SPLIT):
            rso = crso.tile([P, span], f16, tag=f"rso{c}",
                            name=f"rso{lbl}_{c}")
            nc.gpsimd.collective_compute(
                "ReduceScatter", ALU.add, replica_groups=REPLICAS,
                ins=[arins[c].opt()], outs=[rso.opt()])
            nc.sync.dma_start(ysb[:, rof:rof + span], rso[:])
            nc.vector.reduce_sum(out=ysum[:, c:c + 1],
                                 in_=ysb[:, rof:rof + span], axis=AXX)
            scr = ytp.tile([P, 3 * CH], f16, tag="scr", name=f"sq{lbl}_{c}")
            nc.scalar.activation(scr[:, 0:span], ysb[:, rof:rof + span],
                                 AF.Square, accum_out=sqp[:, c:c + 1])
            rof += span
        stot = small.tile([P, 1], f32, tag="stot", name=f"stot{lbl}")
        sqt = small.tile([P, 1], f32, tag="sqt", name=f"sqt{lbl}")
        nc.vector.reduce_sum(out=stot[:], in_=ysum[:], axis=AXX)
        nc.vector.reduce_sum(out=sqt[:], in_=sqp[:], axis=AXX)
        mean = small.tile([P, 1], f32, tag="mean", name=f"mean{lbl}")
        nc.vector.tensor_scalar_mul(mean[:], stot[:], 1.0 / T)
        msq = small.tile([P, 1], f32, tag="msq", name=f"msq{lbl}")
        nc.vector.tensor_tensor(out=msq[:], in0=mean[:], in1=mean[:],
                                op=ALU.mult)
        veps = small.tile([P, 1], f32, tag="veps", name=f"veps{lbl}")
        nc.vector.scalar_tensor_tensor(out=veps[:], in0=sqt[:], scalar=1.0 / T,
                                       in1=msq[:], op0=ALU.mult,
                                       op1=ALU.subtract)
        nc.vector.tensor_scalar_add(veps[:], veps[:], EPS)
        rec = small.tile([P, 1], f32, tag="rec", name=f"rec{lbl}")
        nc.vector.reciprocal(rec[:], veps[:])
        rstd = small.tile([P, 1], f32, tag="rstd", name=f"rstd{lbl}")
        nc.scalar.sqrt(rstd[:], rec[:])
        sca = small.tile([P, 1], f32, tag="sca", name=f"sca{lbl}")
        nc.vector.tensor_tensor(out=sca[:], in0=g_sb[:], in1=rstd[:],
                                op=ALU.mult)
        sh = small.tile([P, 1], f32, tag="sh", name=f"sh{lbl}")
        nc.vector.tensor_tensor(out=sh[:], in0=mean[:], in1=sca[:],
                                op=ALU.mult)
        nc.vector.tensor_tensor(out=sh[:], in0=be_sb[:], in1=sh[:],
                                op=ALU.subtract)
        aof = 0
        for a, span in enumerate(AG_SPLIT):
            asb = ytp.tile([P, 3 * CH], f16, tag="scr", name=f"ap{lbl}_{a}")
            nc.scalar.activation(asb[:, 0:span], ysb[:, aof:aof + span],
                                 AF.Identity,
                                 bias=sh[:, 0:1], scale=sca[:, 0:1])
            agi = cagi.tile([P, span], f16, tag=f"agi{a}",
                            name=f"agi{lbl}_{a}")
            nc.sync.dma_start(agi[:], asb[:, 0:span])
            ago = cago.tile([D, span], f16, tag=f"ago{a}",
                            addr_space="Shared", name=f"ago{lbl}_{a}")
            nc.gpsimd.collective_compute(
                "AllGather", ALU.bypass, replica_groups=REPLICAS,
                ins=[agi.opt()], outs=[ago.opt()])
            # chunked writeback so the consumer can start on the first chunk
            for cc_ in range(span // CH):
                c_ = aof // CH + cc_
                nc.sync.dma_start(
                    xout[:, :, ts(c_, CH)],
                    ago[:, ts(cc_, CH)].rearrange("(k p) t -> p k t", p=P))
            aof += span

    # ---------------------------------------- layers
    for l in range(n_layers):
        # ---- layer weights/params to SBUF
        wq_sb = wqkv.tile([P, DT, DSH], f16, tag="wq", name=f"wq{l}")
        wk_sb = wqkv.tile([P, DT, DSH], f16, tag="wk", name=f"wk{l}")
        wv_sb = wqkv.tile([P, DT, DSH], f16, tag="wv", name=f"wv{l}")
        wo_sbA = wqkv.tile([HD, D], f16, tag="woA", name=f"woA{l}")
        wo_sbB = wqkv.tile([HD, D], f16, tag="woB", name=f"woB{l}")
        w1_sb = wpool.tile([P, DT, FSH], f16, tag="w1", name=f"w1{l}")
        w2_sb = wpool.tile([P, KL, D], f16, tag="w2", name=f"w2{l}")
        nc.sync.dma_start(wq_sb[:], io["wq"][l].rearrange("(k p) m -> p k m", p=P))
        nc.sync.dma_start(wk_sb[:], io["wk"][l].rearrange("(k p) m -> p k m", p=P))
        nc.sync.dma_start(wv_sb[:], io["wv"][l].rearrange("(k p) m -> p k m", p=P))
        nc.sync.dma_start(wo_sbA[:], io["wo"][l][0:HD, :])
        nc.sync.dma_start(wo_sbB[:], io["wo"][l][HD:2 * HD, :])
        nc.sync.dma_start(w1_sb[:], io["w1"][l].rearrange("(k p) m -> p k m", p=P))
        nc.sync.dma_start(w2_sb[:], io["w2"][l].rearrange("(k p) m -> p k m", p=P))

        bq_sb = small.tile([P, 1], f32, tag="bq", name=f"bq{l}")
        bk_sb = small.tile([P, 1], f32, tag="bk", name=f"bk{l}")
        bvr_sb = small.tile([1, DSH], f16, tag="bvr", name=f"bvr{l}")
        b1_sb = small.tile([P, FMT], f32, tag="b1", name=f"b1{l}")
        nc.sync.dma_start(bq_sb[:], io["bq"][l].rearrange("(p o) -> p o", o=1))
        nc.sync.dma_start(bk_sb[:], io["bk"][l].rearrange("(p o) -> p o", o=1))
        nc.sync.dma_start(bvr_sb[:], io["bvr"][l])
        nc.sync.dma_start(b1_sb[:], io["b1"][l].rearrange("(m p) -> p m", p=P))

        g1_sb = small.tile([P, 1], f32, tag="g1", name=f"g1{l}")
        be1_sb = small.tile([P, 1], f32, tag="be1", name=f"be1{l}")
        g2_sb = small.tile([P, 1], f32, tag="g2", name=f"g2{l}")
        be2_sb = small.tile([P, 1], f32, tag="be2", name=f"be2{l}")
        nc.sync.dma_start(g1_sb[:], io["g1s"][l].rearrange("(p o) -> p o", o=1))
        nc.sync.dma_start(be1_sb[:], io["be1s"][l].rearrange("(p o) -> p o", o=1))
        nc.sync.dma_start(g2_sb[:], io["g2s"][l].rearrange("(p o) -> p o", o=1))
        nc.sync.dma_start(be2_sb[:], io["be2s"][l].rearrange("(p o) -> p o", o=1))

        # ---- attention sublayer: QKV + attn + Wo partials, per token chunk
        arins1 = []
        for c in range(NCH):
            # Q^T/K^T shard for chunk c
            psq = ps.tile([P, CH], f32, tag="mm", name=f"psq{l}_{c}")
            psk = ps.tile([P, CH], f32, tag="mm", name=f"psk{l}_{c}")
            for k in range(DT):
                fl, ll = (k == 0), (k == DT - 1)
                rhs = xA[:, k, ts(c, CH)]
                nc.tensor.matmul(psq[:], wq_sb[:, k, :], rhs, start=fl, stop=ll)
                nc.tensor.matmul(psk[:], wk_sb[:, k, :], rhs, start=fl, stop=ll)
            nc.scalar.activation(qT[:, ts(c, CH)], psq[:], AF.Identity,
                                 bias=bq_sb[:])
            nc.scalar.activation(kT[:, ts(c, CH)], psk[:], AF.Identity,
                                 bias=bk_sb[:])
            # V token-major: V_tile = x_tile^T @ Wv + ones^T bvr
            for tt in range(CH // P):
                t = c * (CH // P) + tt
                psv = pst.tile([P, DSH], f32, tag="tv", name=f"psv{l}_{t}")
                for k in range(DT):
                    nc.tensor.matmul(psv[:], xA[:, k, ts(t, P)], wv_sb[:, k, :],
                                     start=(k == 0), stop=False)
                nc.tensor.matmul(psv[:], onesH[0:1, 0:P], bvr_sb[0:1, :],
                                 start=False, stop=True)
                nc.vector.tensor_copy(
                    vsb[:, t, :].rearrange("p (h x) -> p h x", h=2)[:, :, 0:HD],
                    psv[:].rearrange("p (h x) -> p h x", h=2))

            # attention for batch b == c (keys/values = this chunk)
            b = c
            for h, attnT_h in enumerate([attnTA, attnTB]):
                hp = h * HD
                vof = h * (HD + 1)
                ets = []
                for sk in range(B):
                    pss = ps.tile([P, CH], f32, tag="mm",
                                  name=f"pss{l}_{b}_{h}_{sk}")
                    nc.tensor.matmul(
                        pss[:],
                        kT[hp:hp + HD, b * CH + sk * P:b * CH + (sk + 1) * P],
                        qT[hp:hp + HD, ts(b, CH)],
                        start=True, stop=True)
                    et = e512.tile([P, CH], f16, tag="e",
                                   name=f"et{l}_{b}_{h}_{sk}")
                    nc.scalar.activation(et[:], pss[:], AF.Exp,
                                         scale=att_scale)
                    ets.append(et)
                psu = ps.tile([P, CH], f32, tag="mm", name=f"psu{l}_{b}_{h}")
                for sk in range(B):
                    nc.tensor.matmul(psu[0:HD + 1, :],
                                     vsb[:, b * 4 + sk, vof:vof + HD + 1],
                                     ets[sk][:],
                                     start=(sk == 0), stop=(sk == B - 1))
                rss = rp.tile([1, CH], f32, tag="rss", name=f"rss{l}_{b}_{h}")
                nc.scalar.copy(rss[:], psu[HD:HD + 1, :])
                rsb = rp.tile([1, CH], f32, tag="r32", name=f"rsb{l}_{b}_{h}")
                nc.vector.reciprocal_approx_fast(rsb[:], rss[:])
                rsb16 = rp.tile([1, CH], f16, tag="r16", name=f"rsc{l}_{b}_{h}")
                nc.scalar.copy(rsb16[:], rsb[:])
                psr = ps.tile([P, CH], f32, tag="mm", name=f"psr{l}_{b}_{h}")
                nc.tensor.matmul(psr[0:HD, :], onesH[0:1, 0:HD], rsb16[:],
                                 start=True, stop=True)
                usb = e512.tile([P, CH], f16, tag="e", name=f"usb{l}_{b}_{h}")
                nc.scalar.copy(usb[0:HD, :], psu[0:HD, :])
                nc.vector.tensor_tensor(out=attnT_h[:, ts(b, CH)],
                                        in0=usb[0:HD, :],
                                        in1=psr[0:HD, :], op=ALU.mult)

            # Wo partial + residual/8 for chunk c
            ri, cof = rs_chunk_map[c]
            if cof == 0:
                arin = crin.tile([D, RS_SPLIT[ri]], f16, tag=f"ari{ri}",
                                 name=f"ari1_{l}_{c}")
                arins1.append(arin)
            for m in range(DT):
                ps2 = ps.tile([P, CH], f32, tag="mm", name=f"pso{l}_{m}_{c}")
                nc.tensor.matmul(ps2[:], wo_sbA[:, ts(m, P)],
                                 attnTA[:, ts(c, CH)], start=True, stop=False)
                nc.tensor.matmul(ps2[:], wo_sbB[:, ts(m, P)],
                                 attnTB[:, ts(c, CH)], start=False, stop=True)
                osb = e512.tile([P, CH], f16, tag="e", name=f"osb{l}_{m}_{c}")
                nc.vector.scalar_tensor_tensor(
                    out=osb[:], in0=xA[:, m, ts(c, CH)], scalar=1.0 / NC,
                    in1=ps2[:], op0=ALU.mult, op1=ALU.add)
                nc.sync.dma_start(arin[ts(m, P), cof:cof + CH], osb[:])

        # ---- BN1 -> x2 (xB)
        bn_sublayer(f"a{l}", arins1, g1_sb, be1_sb, xB)

        # ---- FFN sublayer, per token chunk
        arins2 = []
        for c in range(NCH):
            ht = htp.tile([P, FMT, CH], f16, tag="ht", name=f"ht{l}_{c}")
            for m in range(FMT):
                ps1 = ps.tile([P, CH], f32, tag="mm", name=f"ps1{l}_{c}_{m}")
                for k in range(DT):
                    nc.tensor.matmul(ps1[:], w1_sb[:, k, ts(m, P)],
                                     xB[:, k, ts(c, CH)],
                                     start=(k == 0), stop=(k == DT - 1))
                nc.scalar.activation(ht[:, m, :], ps1[:], AF.Relu,
                                     bias=b1_sb[:, m:m + 1])
            ri, cof = rs_chunk_map[c]
            if cof == 0:
                arin = crin.tile([D, RS_SPLIT[ri]], f16, tag=f"ari{ri}",
                                 name=f"ari2_{l}_{c}")
                arins2.append(arin)
            for m in range(DT):
                ps2 = ps.tile([P, CH], f32, tag="mm", name=f"ps2{l}_{c}_{m}")
                for k in range(KL):
                    nc.tensor.matmul(ps2[:], w2_sb[:, k, ts(m, P)],
                                     ht[:, k, :], start=(k == 0),
                                     stop=(k == KL - 1))
                osb = e512.tile([P, CH], f16, tag="e", name=f"fsb{l}_{c}_{m}")
                nc.vector.scalar_tensor_tensor(
                    out=osb[:], in0=xB[:, m, ts(c, CH)], scalar=1.0 / NC,
                    in1=ps2[:], op0=ALU.mult, op1=ALU.add)
                nc.sync.dma_start(arin[ts(m, P), cof:cof + CH], osb[:])

        # ---- BN2 -> x(l+1) (xA)
        bn_sublayer(f"f{l}", arins2, g2_sb, be2_sb, xA)

    # ---------------------------------------- output x^T -> [D, T] f16
    nc.sync.dma_start(io["out"].rearrange("(k p) t -> p k t", p=P), xA[:])
    st.close()


# ================================================================ host side

def make_in_maps(inputs):
    f = lambda a: np.ascontiguousarray(np.asarray(a), dtype=np.float32)
    h = lambda a: np.ascontiguousarray(np.asarray(a), dtype=np.float16)
    seq = np.asarray(inputs["sequence"]).reshape(-1).astype(np.int16)
    idx = np.ascontiguousarray(seq.reshape(T // 16, 16).T)     # [16, T//16]
    emb = h(inputs["emb"])
    pesT = np.ascontiguousarray(h(inputs["pes"]).T)            # [D, S]
    Wq, Wk, Wv = h(inputs["Wq"]), h(inputs["Wk"]), h(inputs["Wv"])
    Wo, W1, W2 = h(inputs["Wo"]), h(inputs["W1"]), h(inputs["W2"])
    bq, bk, bv = f(inputs["bq"]), f(inputs["bk"]), f(inputs["bv"])
    b1 = f(inputs["b1"])
    g1, be1 = f(inputs["g1"]), f(inputs["be1"])
    g2, be2 = f(inputs["g2"]), f(inputs["be2"])

    in_maps = []
    for c in range(NC):
        ds_ = slice(c * DSH, (c + 1) * DSH)
        fs_ = slice(c * FSH, (c + 1) * FSH)
        cs_ = slice(c * P, (c + 1) * P)
        in_maps.append({
            "emb": emb,
            "idx": idx,
            "pesT": pesT,
            "wq": np.ascontiguousarray(Wq[:, :, ds_]),
            "wk": np.ascontiguousarray(Wk[:, :, ds_]),
            "wv": np.ascontiguousarray(Wv[:, :, ds_]),
            "wo": np.ascontiguousarray(Wo[:, ds_, :]),
            "w1": np.ascontiguousarray(W1[:, :, fs_]),
            "w2": np.ascontiguousarray(W2[:, fs_, :]),
            "bq": np.ascontiguousarray(bq[:, ds_]),
            "bk": np.ascontiguousarray(bk[:, ds_]),
            "bvr": np.ascontiguousarray(bv[:, ds_]).astype(np.float16)
                     .reshape(L, 1, DSH),
            "b1": np.ascontiguousarray(b1[:, fs_]),
            "g1s": np.ascontiguousarray(g1[:, cs_]),
            "be1s": np.ascontiguousarray(be1[:, cs_]),
            "g2s": np.ascontiguousarray(g2[:, cs_]),
            "be2s": np.ascontiguousarray(be2[:, cs_]),
        })
    return in_maps


_CACHE = {}


def _get_module():
    if "nc" not in _CACHE:
        _CACHE["nc"] = build_module()
    return _CACHE["nc"]


def kernel(**inputs):
    from concourse import bass_utils
    nc = _get_module()
    in_maps = make_in_maps(inputs)
    res = bass_utils.run_bass_kernel_spmd(nc, in_maps, list(range(NC)))
    o = np.asarray(res.results[0]["out"])                  # [D, T]
    return np.ascontiguousarray(o.T).reshape(B, S, D).astype(np.float32)


# revision 43
# speedup vs baseline: 1.1195x; 1.0024x over previous
"""Trainium2 Bass kernel for a 6-layer post-BatchNorm transformer encoder.

Reference model:
  x = emb[seq] + pes                                  # [B,S,D] = [4,512,1024]
  6x: x = BN(x + attn(x)); x = BN(x + ffn(x))
  BN = per-channel batch stats over (B,S), eps=1e-3.

Sharding: tensor-parallel across 8 NeuronCores (2 heads + 512 FFN hidden per
core). v2 replaces the fp32 AllReduce + redundant-BN design with a chunked
fp16 ReduceScatter -> per-core BN on a 128-channel slice -> chunked fp16
AllGather pipeline: each sublayer's partial output is written in 4 token
chunks (512 tokens = one batch element each); RS chunk c overlaps with
compute of chunk c+1, BN stats accumulate per chunk as RS results land, and
after the affine is finalized the AG chunks stream back while the next
sublayer's matmuls consume them chunk-by-chunk. The residual x/8 is folded
into each partial via a fused DVE scalar_tensor_tensor (no extra PE matmul).

All activations and weights are fp16 (PSUM accumulation fp32; softmax
normalization and BN statistics fp32). Numpy emulation of this exact
quantization scheme gives max rel err ~2.5e-3 vs the fp32 reference
(tolerance 2e-2).

Activation layout: transposed. x^T lives in SBUF as [128 part, 8 dtile,
2048 tok]. Embedding uses dma_gather(transpose=True) which delivers rows
directly in x^T layout (no PE transposes). V is produced token-major by
swapping stationary/moving in the matmul (V_tile = x_tile^T @ Wv), with the
bias added via a ones-row rank-1 matmul, so no V transposes either.
"""

import os

import numpy as np

import concourse.bass as bass
import concourse.mybir as mybir
import concourse.tile as tile
from concourse import bacc
from concourse.bass import ts
from concourse.masks import make_identity

# ---------------------------------------------------------------- dims
V, D, L, H, B, S = 32000, 1024, 6, 16, 4, 512
HD = D // H            # 64
DF = 4 * D             # 4096
EPS = 1e-3
NC = 8                 # cores
T = B * S              # 2048 tokens
P = 128                # partitions
DT = D // P            # 8 d-tiles
TT = T // P            # 16 token tiles
CH = 512               # token chunk (matmul N) == S
NCH = T // CH          # 4 chunks == B
HPC = H // NC          # heads per core = 2
DSH = HPC * HD         # qkv out shard = 128
FSH = DF // NC         # ffn hidden shard = 512
FMT = FSH // P         # ffn1 m-tiles = 4
KL = FSH // P          # ffn2 k-tiles = 4

f32 = mybir.dt.float32
f16 = mybir.dt.float16
f32r = mybir.dt.float32r
i16 = mybir.dt.int16
AF = mybir.ActivationFunctionType
ALU = mybir.AluOpType
AXX = mybir.AxisListType.X

REPLICAS = [list(range(NC))]

N_LAYERS = int(os.environ.get("TRN_KERNEL_LAYERS", str(L)))
GATHER_QUEUES = int(os.environ.get("TRN_GATHER_QUEUES", "1"))


def build_module(n_layers=None):
    if n_layers is None:
        n_layers = N_LAYERS
    nc = bacc.Bacc("TRN2", target_bir_lowering=False, debug=False,
                   num_devices=NC)

    dt_ = nc.dram_tensor
    io = {
        "emb": dt_("emb", [V, D], f16, kind="ExternalInput").ap(),
        "idx": dt_("idx", [16, T // 16], i16, kind="ExternalInput").ap(),
        "pesT": dt_("pesT", [D, S], f16, kind="ExternalInput").ap(),
        "wq": dt_("wq", [L, D, DSH], f16, kind="ExternalInput").ap(),
        "wk": dt_("wk", [L, D, DSH], f16, kind="ExternalInput").ap(),
        "wv": dt_("wv", [L, D, DSH], f16, kind="ExternalInput").ap(),
        "wo": dt_("wo", [L, DSH, D], f16, kind="ExternalInput").ap(),
        "w1": dt_("w1", [L, D, FSH], f16, kind="ExternalInput").ap(),
        "w2": dt_("w2", [L, FSH, D], f16, kind="ExternalInput").ap(),
        "bq": dt_("bq", [L, DSH], f32, kind="ExternalInput").ap(),
        "bk": dt_("bk", [L, DSH], f32, kind="ExternalInput").ap(),
        "bvr": dt_("bvr", [L, 1, DSH], f16, kind="ExternalInput").ap(),
        "b1": dt_("b1", [L, FSH], f32, kind="ExternalInput").ap(),
        "g1s": dt_("g1s", [L, P], f32, kind="ExternalInput").ap(),
        "be1s": dt_("be1s", [L, P], f32, kind="ExternalInput").ap(),
        "g2s": dt_("g2s", [L, P], f32, kind="ExternalInput").ap(),
        "be2s": dt_("be2s", [L, P], f32, kind="ExternalInput").ap(),
        "out": dt_("out", [D, T], f16, kind="ExternalOutput").ap(),
    }

    with tile.TileContext(nc) as tc:
        _build(tc, n_layers, io)
    nc.compile()
    return nc


def _build(tc, n_layers, io):
    from contextlib import ExitStack
    nc = tc.nc
    att_scale = 1.0 / np.sqrt(HD)

    # ------------------------------------------------ pools
    st = ExitStack()
    persist = st.enter_context(tc.tile_pool(name="persist", bufs=1))
    wpool = st.enter_context(tc.tile_pool(name="wpool", bufs=2))   # W1/W2
    wqkv = st.enter_context(tc.tile_pool(name="wqkv", bufs=2))     # Wq/Wk/Wv/Wo
    small = st.enter_context(tc.tile_pool(name="small", bufs=2))   # biases/stats
    ytp = st.enter_context(tc.tile_pool(name="ytp", bufs=2))       # RS result
    e512 = st.enter_context(tc.tile_pool(name="e512", bufs=10))    # [128, CH] f16
    rp = st.enter_context(tc.tile_pool(name="rp", bufs=2))         # f32 recips
    htp = st.enter_context(tc.tile_pool(name="htp", bufs=2))       # ffn hidden

    ps = st.enter_context(tc.tile_pool(name="ps", bufs=5, space="PSUM"))
    pst = st.enter_context(tc.tile_pool(name="pst", bufs=2, space="PSUM"))
    crin = st.enter_context(tc.tile_pool(name="crin", bufs=8, space="DRAM"))
    crso = st.enter_context(tc.tile_pool(name="crso", bufs=8, space="DRAM"))
    cagi = st.enter_context(tc.tile_pool(name="cagi", bufs=8, space="DRAM"))
    cago = st.enter_context(tc.tile_pool(name="cago", bufs=8, space="DRAM"))

    # ------------------------------------------------ persistent tiles
    xA = persist.tile([P, DT, T], f16, name="xA")          # x
    xB = persist.tile([P, DT, T], f16, name="xB")          # x2
    qT = persist.tile([P, T], f16, name="qT")              # Q^T shard
    kT = persist.tile([P, T], f16, name="kT")              # K^T shard
    vsb = persist.tile([P, TT, 2 * (HD + 1)], f16, name="vsb")  # [V|1|V|1]
    onesH = persist.tile([P, P], f16, name="onesH")
    attnTA = persist.tile([HD, T], f16, name="attnTA")     # head-0 attn^T
    attnTB = persist.tile([HD, T], f16, name="attnTB")     # head-1 attn^T
    pes_sb = persist.tile([P, DT, S], f16, name="pes_sb")  # pes^T
    idxs = persist.tile([P, T // 16], i16, name="idxs")

    nc.vector.memset(onesH[:], 1.0)
    # ones columns in vsb (positions HD and 2*HD+1 of each token tile)
    nc.vector.memset(vsb[:, :, HD:HD + 1], 1.0)
    nc.vector.memset(vsb[:, :, 2 * HD + 1:], 1.0)
    # indices wrapped in 16 partitions, replicated into all 8 Q7-core stripes
    for r_ in range(P // 16):
        nc.sync.dma_start(idxs[16 * r_:16 * (r_ + 1), :], io["idx"])
    nc.sync.dma_start(pes_sb[:], io["pesT"].rearrange("(k p) s -> p k s", p=P))

    # ---------------- embedding: x^T via transposing gather, + pes^T
    for c in range(NCH):
        gt = ytp.tile([P, DT, CH], f16, tag="gt", name=f"gt{c}")
        nc.gpsimd.dma_gather(
            out_ap=gt[:],
            in_ap=io["emb"],
            idxs_ap=idxs[:, c * (CH // 16):(c + 1) * (CH // 16)],
            num_idxs=CH,
            num_idxs_reg=CH,
            elem_size=D,
            transpose=True,
            queue_num=c % GATHER_QUEUES,
        )
        nc.vector.tensor_tensor(out=xA[:, :, ts(c, CH)],
                                in0=gt[:],
                                in1=pes_sb[:], op=ALU.add)

    # ---------------- chunked RS -> per-core-slice BN -> chunked AG
    # Asymmetric splits: big RS first so the last (exposed) RS is small;
    # small AG first so the consumer's first chunk arrives early.
    RS_SPLIT = [CH, CH, CH, CH]     # token spans per ReduceScatter
    AG_SPLIT = [CH, CH, 2 * CH]     # token spans per AllGather
    NRS = len(RS_SPLIT)
    rs_chunk_map = {}               # token chunk -> (rs index, col offset)
    _c = 0
    for _ri, _span in enumerate(RS_SPLIT):
        for _j in range(_span // CH):
            rs_chunk_map[_c] = (_ri, _j * CH)
            _c += 1

    def bn_sublayer(lbl, arins, g_sb, be_sb, xout):
        """arins: list of written [D, span] f16 DRAM tiles (partial + x/8).
        Reduces across cores, BN-normalizes this core's 128-channel slice,
        gathers normalized chunks into xout ([P, DT, T] f16)."""
        ysb = ytp.tile([P, T], f16, tag="ysb", name=f"ysb{lbl}")
        ysum = small.tile([P, NRS], f32, tag="ysum", name=f"ysum{lbl}")
        sqp = small.tile([P, NRS], f32, tag="sqp", name=f"sqp{lbl}")
        rof = 0
        for c, span in enumerate(RS_SPLIT):
            rso = crso.tile([P, span], f16, tag=f"rso{c}",
                            name=f"rso{lbl}_{c}")
            nc.gpsimd.collective_compute(
                "ReduceScatter", ALU.add, replica_groups=REPLICAS,
                ins=[arins[c].opt()], outs=[rso.opt()])
            nc.sync.dma_start(ysb[:, rof:rof + span], rso[:])
            nc.vector.reduce_sum(out=ysum[:, c:c + 1],
                                 in_=ysb[:, rof:rof + span], axis=AXX)
            scr = ytp.tile([P, 3 * CH], f16, tag="scr", name=f"sq{lbl}_{c}")
            nc.scalar.activation(scr[:, 0:span], ysb[:, rof:rof + span],
                                 AF.Square, accum_out=sqp[:, c:c + 1])
            rof += span
        stot = small.tile([P, 1], f32, tag="stot", name=f"stot{lbl}")
        sqt = small.tile([P, 1], f32, tag="sqt", name=f"sqt{lbl}")
        nc.vector.reduce_sum(out=stot[:], in_=ysum[:], axis=AXX)
        nc.vector.reduce_sum(out=sqt[:], in_=sqp[:], axis=AXX)
        mean = small.tile([P, 1], f32, tag="mean", name=f"mean{lbl}")
        nc.vector.tensor_scalar_mul(mean[:], stot[:], 1.0 / T)
        msq = small.tile([P, 1], f32, tag="msq", name=f"msq{lbl}")
        nc.vector.tensor_tensor(out=msq[:], in0=mean[:], in1=mean[:],
                                op=ALU.mult)
        veps = small.tile([P, 1], f32, tag="veps", name=f"veps{lbl}")
        nc.vector.scalar_tensor_tensor(out=veps[:], in0=sqt[:], scalar=1.0 / T,
                                       in1=msq[:], op0=ALU.mult,
                                       op1=ALU.subtract)
        nc.vector.tensor_scalar_add(veps[:], veps[:], EPS)
        rec = small.tile([P, 1], f32, tag="rec", name=f"rec{lbl}")
        nc.vector.reciprocal(rec[:], veps[:])
        rstd = small.tile([P, 1], f32, tag="rstd", name=f"rstd{lbl}")
        nc.scalar.sqrt(rstd[:], rec[:])
        sca = small.tile([P, 1], f32, tag="sca", name=f"sca{lbl}")
        nc.vector.tensor_tensor(out=sca[:], in0=g_sb[:], in1=rstd[:],
                                op=ALU.mult)
        sh = small.tile([P, 1], f32, tag="sh", name=f"sh{lbl}")
        nc.vector.tensor_tensor(out=sh[:], in0=mean[:], in1=sca[:],
                                op=ALU.mult)
        nc.vector.tensor_tensor(out=sh[:], in0=be_sb[:], in1=sh[:],
                                op=ALU.subtract)
        aof = 0
        for a, span in enumerate(AG_SPLIT):
            asb = ytp.tile([P, 3 * CH], f16, tag="scr", name=f"ap{lbl}_{a}")
            nc.scalar.activation(asb[:, 0:span], ysb[:, aof:aof + span],
                                 AF.Identity,
                                 bias=sh[:, 0:1], scale=sca[:, 0:1])
            agi = cagi.tile([P, span], f16, tag=f"agi{a}",
                            name=f"agi{lbl}_{a}")
            nc.sync.dma_start(agi[:], asb[:, 0:span])
            ago = cago.tile([D, span], f16, tag=f"ago{a}",
                            addr_space="Shared", name=f"ago{lbl}_{a}")
            nc.gpsimd.collective_compute(
                "AllGather", ALU.bypass, replica_groups=REPLICAS,
                ins=[agi.opt()], outs=[ago.opt()])
            # chunked writeback so the consumer can start on the first chunk
            for cc_ in range(span // CH):
                c_ = aof // CH + cc_
                nc.sync.dma_start(
                    xout[:, :, ts(c_, CH)],
                    ago[:, ts(cc_, CH)].rearrange("(k p) t -> p k t", p=P))
            aof += span

    # ---------------------------------------- layers
    for l in range(n_layers):
        # ---- layer weights/params to SBUF
        wq_sb = wqkv.tile([P, DT, DSH], f16, tag="wq", name=f"wq{l}")
        wk_sb = wqkv.tile([P, DT, DSH], f16, tag="wk", name=f"wk{l}")
        wv_sb = wqkv.tile([P, DT, DSH], f16, tag="wv", name=f"wv{l}")
        wo_sbA = wqkv.tile([HD, D], f16, tag="woA", name=f"woA{l}")
        wo_sbB = wqkv.tile([HD, D], f16, tag="woB", name=f"woB{l}")
        w1_sb = wpool.tile([P, DT, FSH], f16, tag="w1", name=f"w1{l}")
        w2_sb = wpool.tile([P, KL, D], f16, tag="w2", name=f"w2{l}")
        nc.sync.dma_start(wq_sb[:], io["wq"][l].rearrange("(k p) m -> p k m", p=P))
        nc.sync.dma_start(wk_sb[:], io["wk"][l].rearrange("(k p) m -> p k m", p=P))
        nc.sync.dma_start(wv_sb[:], io["wv"][l].rearrange("(k p) m -> p k m", p=P))
        nc.sync.dma_start(wo_sbA[:], io["wo"][l][0:HD, :])
        nc.sync.dma_start(wo_sbB[:], io["wo"][l][HD:2 * HD, :])
        nc.sync.dma_start(w1_sb[:], io["w1"][l].rearrange("(k p) m -> p k m", p=P))
        nc.sync.dma_start(w2_sb[:], io["w2"][l].rearrange("(k p) m -> p k m", p=P))

        bq_sb = small.tile([P, 1], f32, tag="bq", name=f"bq{l}")
        bk_sb = small.tile([P, 1], f32, tag="bk", name=f"bk{l}")
        bvr_sb = small.tile([1, DSH], f16, tag="bvr", name=f"bvr{l}")
        b1_sb = small.tile([P, FMT], f32, tag="b1", name=f"b1{l}")
        nc.sync.dma_start(bq_sb[:], io["bq"][l].rearrange("(p o) -> p o", o=1))
        nc.sync.dma_start(bk_sb[:], io["bk"][l].rearrange("(p o) -> p o", o=1))
        nc.sync.dma_start(bvr_sb[:], io["bvr"][l])
        nc.sync.dma_start(b1_sb[:], io["b1"][l].rearrange("(m p) -> p m", p=P))

        g1_sb = small.tile([P, 1], f32, tag="g1", name=f"g1{l}")
        be1_sb = small.tile([P, 1], f32, tag="be1", name=f"be1{l}")
        g2_sb = small.tile([P, 1], f32, tag="g2", name=f"g2{l}")
        be2_sb = small.tile([P, 1], f32, tag="be2", name=f"be2{l}")
        nc.sync.dma_start(g1_sb[:], io["g1s"][l].rearrange("(p o) -> p o", o=1))
        nc.sync.dma_start(be1_sb[:], io["be1s"][l].rearrange("(p o) -> p o", o=1))
        nc.sync.dma_start(g2_sb[:], io["g2s"][l].rearrange("(p o) -> p o", o=1))
        nc.sync.dma_start(be2_sb[:], io["be2s"][l].rearrange("(p o) -> p o", o=1))

        # ---- attention sublayer: QKV + attn + Wo partials, per token chunk
        arins1 = []
        for c in range(NCH):
            # Q^T/K^T shard for chunk c
            psq = ps.tile([P, CH], f32, tag="mm", name=f"psq{l}_{c}")
            psk = ps.tile([P, CH], f32, tag="mm", name=f"psk{l}_{c}")
            for k in range(DT):
                fl, ll = (k == 0), (k == DT - 1)
                rhs = xA[:, k, ts(c, CH)]
                nc.tensor.matmul(psq[:], wq_sb[:, k, :], rhs, start=fl, stop=ll)
                nc.tensor.matmul(psk[:], wk_sb[:, k, :], rhs, start=fl, stop=ll)
            nc.scalar.activation(qT[:, ts(c, CH)], psq[:], AF.Identity,
                                 bias=bq_sb[:])
            nc.scalar.activation(kT[:, ts(c, CH)], psk[:], AF.Identity,
                                 bias=bk_sb[:])
            # V token-major: V_tile = x_tile^T @ Wv + ones^T bvr
            for tt in range(CH // P):
                t = c * (CH // P) + tt
                psv = pst.tile([P, DSH], f32, tag="tv", name=f"psv{l}_{t}")
                for k in range(DT):
                    nc.tensor.matmul(psv[:], xA[:, k, ts(t, P)], wv_sb[:, k, :],
                                     start=(k == 0), stop=False)
                nc.tensor.matmul(psv[:], onesH[0:1, 0:P], bvr_sb[0:1, :],
                                 start=False, stop=True)
                nc.vector.tensor_copy(
                    vsb[:, t, :].rearrange("p (h x) -> p h x", h=2)[:, :, 0:HD],
                    psv[:].rearrange("p (h x) -> p h x", h=2))

            # attention for batch b == c (keys/values = this chunk)
            b = c
            for h, attnT_h in enumerate([attnTA, attnTB]):
                hp = h * HD
                vof = h * (HD + 1)
                ets = []
                for sk in range(B):
                    pss = ps.tile([P, CH], f32, tag="mm",
                                  name=f"pss{l}_{b}_{h}_{sk}")
                    nc.tensor.matmul(
                        pss[:],
                        kT[hp:hp + HD, b * CH + sk * P:b * CH + (sk + 1) * P],
                        qT[hp:hp + HD, ts(b, CH)],
                        start=True, stop=True)
                    et = e512.tile([P, CH], f16, tag="e",
                                   name=f"et{l}_{b}_{h}_{sk}")
                    nc.scalar.activation(et[:], pss[:], AF.Exp,
                                         scale=att_scale)
                    ets.append(et)
                psu = ps.tile([P, CH], f32, tag="mm", name=f"psu{l}_{b}_{h}")
                for sk in range(B):
                    nc.tensor.matmul(psu[0:HD + 1, :],
                                     vsb[:, b * 4 + sk, vof:vof + HD + 1],
                                     ets[sk][:],
                                     start=(sk == 0), stop=(sk == B - 1))
                rss = rp.tile([1, CH], f32, tag="rss", name=f"rss{l}_{b}_{h}")
                nc.scalar.copy(rss[:], psu[HD:HD + 1, :])
                rsb = rp.tile([1, CH], f32, tag="r32", name=f"rsb{l}_{b}_{h}")
                nc.vector.reciprocal_approx_fast(rsb[:], rss[:])
                rsb16 = rp.tile([1, CH], f16, tag="r16", name=f"rsc{l}_{b}_{h}")
                nc.scalar.copy(rsb16[:], rsb[:])
                psr = ps.tile([P, CH], f32, tag="mm", name=f"psr{l}_{b}_{h}")
                nc.tensor.matmul(psr[0:HD, :], onesH[0:1, 0:HD], rsb16[:],
                                 start=True, stop=True)
                usb = e512.tile([P, CH], f16, tag="e", name=f"usb{l}_{b}_{h}")
                nc.scalar.copy(usb[0:HD, :], psu[0:HD, :])
                nc.vector.tensor_tensor(out=attnT_h[:, ts(b, CH)],
                                        in0=usb[0:HD, :],
                                        in1=psr[0:HD, :], op=ALU.mult)

            # Wo partial + residual/8 for chunk c
            ri, cof = rs_chunk_map[c]
            if cof == 0:
                arin = crin.tile([D, RS_SPLIT[ri]], f16, tag=f"ari{ri}",
                                 name=f"ari1_{l}_{c}")
                arins1.append(arin)
            for m in range(DT):
                ps2 = ps.tile([P, CH], f32, tag="mm", name=f"pso{l}_{m}_{c}")
                nc.tensor.matmul(ps2[:], wo_sbA[:, ts(m, P)],
                                 attnTA[:, ts(c, CH)], start=True, stop=False)
                nc.tensor.matmul(ps2[:], wo_sbB[:, ts(m, P)],
                                 attnTB[:, ts(c, CH)], start=False, stop=True)
                osb = e512.tile([P, CH], f16, tag="e", name=f"osb{l}_{m}_{c}")
                nc.vector.scalar_tensor_tensor(
                    out=osb[:], in0=xA[:, m, ts(c, CH)], scalar=1.0 / NC,
                    in1=ps2[:], op0=ALU.mult, op1=ALU.add)
                nc.sync.dma_start(arin[ts(m, P), cof:cof + CH], osb[:])

        # ---- BN1 -> x2 (xB)
        bn_sublayer(f"a{l}", arins1, g1_sb, be1_sb, xB)

        # ---- FFN sublayer, per token chunk
        arins2 = []
        for c in range(NCH):
            ht = htp.tile([P, FMT, CH], f16, tag="ht", name=f"ht{l}_{c}")
            for m in range(FMT):
                ps1 = ps.tile([P, CH], f32, tag="mm", name=f"ps1{l}_{c}_{m}")
                for k in range(DT):
                    nc.tensor.matmul(ps1[:], w1_sb[:, k, ts(m, P)],
                                     xB[:, k, ts(c, CH)],
                                     start=(k == 0), stop=(k == DT - 1))
                nc.scalar.activation(ht[:, m, :], ps1[:], AF.Relu,
                                     bias=b1_sb[:, m:m + 1])
            ri, cof = rs_chunk_map[c]
            if cof == 0:
                arin = crin.tile([D, RS_SPLIT[ri]], f16, tag=f"ari{ri}",
                                 name=f"ari2_{l}_{c}")
                arins2.append(arin)
            for m in range(DT):
                ps2 = ps.tile([P, CH], f32, tag="mm", name=f"ps2{l}_{c}_{m}")
                for k in range(KL):
                    nc.tensor.matmul(ps2[:], w2_sb[:, k, ts(m, P)],
                                     ht[:, k, :], start=(k == 0),
                                     stop=(k == KL - 1))
                osb = e512.tile([P, CH], f16, tag="e", name=f"fsb{l}_{c}_{m}")
                nc.vector.scalar_tensor_tensor(
                    out=osb[:], in0=xB[:, m, ts(c, CH)], scalar=1.0 / NC,
                    in1=ps2[:], op0=ALU.mult, op1=ALU.add)
                nc.sync.dma_start(arin[ts(m, P), cof:cof + CH], osb[:])

        # ---- BN2 -> x(l+1) (xA)
        bn_sublayer(f"f{l}", arins2, g2_sb, be2_sb, xA)

    # ---------------------------------------- output x^T -> [D, T] f16
    nc.sync.dma_start(io["out"].rearrange("(k p) t -> p k t", p=P), xA[:])
    st.close()


# ================================================================ host side

def make_in_maps(inputs):
    f = lambda a: np.ascontiguousarray(np.asarray(a), dtype=np.float32)
    h = lambda a: np.ascontiguousarray(np.asarray(a), dtype=np.float16)
    seq = np.asarray(inputs["sequence"]).reshape(-1).astype(np.int16)
    idx = np.ascontiguousarray(seq.reshape(T // 16, 16).T)     # [16, T//16]
    emb = h(inputs["emb"])
    pesT = np.ascontiguousarray(h(inputs["pes"]).T)            # [D, S]
    Wq, Wk, Wv = h(inputs["Wq"]), h(inputs["Wk"]), h(inputs["Wv"])
    Wo, W1, W2 = h(inputs["Wo"]), h(inputs["W1"]), h(inputs["W2"])
    bq, bk, bv = f(inputs["bq"]), f(inputs["bk"]), f(inputs["bv"])
    b1 = f(inputs["b1"])
    g1, be1 = f(inputs["g1"]), f(inputs["be1"])
    g2, be2 = f(inputs["g2"]), f(inputs["be2"])

    in_maps = []
    for c in range(NC):
        ds_ = slice(c * DSH, (c + 1) * DSH)
        fs_ = slice(c * FSH, (c + 1) * FSH)
        cs_ = slice(c * P, (c + 1) * P)
        in_maps.append({
            "emb": emb,
            "idx": idx,
            "pesT": pesT,
            "wq": np.ascontiguousarray(Wq[:, :, ds_]),
            "wk": np.ascontiguousarray(Wk[:, :, ds_]),
            "wv": np.ascontiguousarray(Wv[:, :, ds_]),
            "wo": np.ascontiguousarray(Wo[:, ds_, :]),
            "w1": np.ascontiguousarray(W1[:, :, fs_]),
            "w2": np.ascontiguousarray(W2[:, fs_, :]),
            "bq": np.ascontiguousarray(bq[:, ds_]),
            "bk": np.ascontiguousarray(bk[:, ds_]),
            "bvr": np.ascontiguousarray(bv[:, ds_]).astype(np.float16)
                     .reshape(L, 1, DSH),
            "b1": np.ascontiguousarray(b1[:, fs_]),
            "g1s": np.ascontiguousarray(g1[:, cs_]),
            "be1s": np.ascontiguousarray(be1[:, cs_]),
            "g2s": np.ascontiguousarray(g2[:, cs_]),
            "be2s": np.ascontiguousarray(be2[:, cs_]),
        })
    return in_maps


_CACHE = {}


def _get_module():
    if "nc" not in _CACHE:
        _CACHE["nc"] = build_module()
    return _CACHE["nc"]


def kernel(**inputs):
    from concourse import bass_utils
    nc = _get_module()
    in_maps = make_in_maps(inputs)
    res = bass_utils.run_bass_kernel_spmd(nc, in_maps, list(range(NC)))
    o = np.asarray(res.results[0]["out"])                  # [D, T]
    return np.ascontiguousarray(o.T).reshape(B, S, D).astype(np.float32)
